# revision 2
# baseline (speedup 1.0000x reference)
"""Trainium2 Bass kernel for the ComplexMixture density-matrix problem.

Math (per batch b):
    out_r[b] = (w*R)^T @ R + (w*I)^T @ I
    out_i[b] = (w*I)^T @ R - (w*R)^T @ I
with R = input_real[b] [S, D], I = input_imag[b] [S, D], w = weight[b] [S].
Contraction is over S, which maps directly onto the PE array's partition
(K) dimension -- no transposes needed anywhere.

Implemented with the 3-multiplication (Karatsuba/Gauss) complex product:
    P1 = WR^T @ R,  Q2 = WI^T @ I,  P3 = (WR+WI)^T @ (R-I)
    out_r = P1 + Q2
    out_i = P3 - P1 + Q2
where WR = w*R, WI = w*I.  This does 3 big matmuls per batch instead of 4.

Sharding: data-parallel over the batch dim B=16 across 8 NeuronCores
(2 batches per core).  No collectives.

Compute dtype: bf16 operands with fp32 PSUM accumulation (bf16 matmul is
4x the fp32 rate on the TRN2 PE array).
"""

import sys

if "/opt/trn_rl_repo" not in sys.path:
    sys.path.insert(0, "/opt/trn_rl_repo")

import numpy as np

# Problem constants (hardcoded per harness contract)
B, S, D = 16, 1024, 768
N_CORES = 8
BPC = B // N_CORES  # batches per core
P = 128  # partitions
KT = S // P  # 8 k-tiles of 128 along S
MT = D // P  # 6 m-tiles of 128 along output rows
NW = 384  # psum tile width (fits one PSUM bank, 2 tiles per D)
NT = D // NW

_PROGRAM = None


def _build_program():
    import concourse.mybir as mybir
    import concourse.tile as tile
    from concourse import bacc

    f32 = mybir.dt.float32
    bf16 = mybir.dt.bfloat16

    nc = bacc.Bacc("TRN2", target_bir_lowering=False, debug=False,
                   num_devices=N_CORES)

    r_dram = nc.dram_tensor("input_real", [BPC, S, D], f32, kind="ExternalInput")
    i_dram = nc.dram_tensor("input_imag", [BPC, S, D], f32, kind="ExternalInput")
    w_dram = nc.dram_tensor("weight", [BPC, S], f32, kind="ExternalInput")
    or_dram = nc.dram_tensor("out_r", [BPC, D, D], f32, kind="ExternalOutput")
    oi_dram = nc.dram_tensor("out_i", [BPC, D, D], f32, kind="ExternalOutput")

    with tile.TileContext(nc) as tc:
        with (
            tc.tile_pool(name="wpool", bufs=1) as wpool,
            tc.tile_pool(name="stage", bufs=3) as stage,
            tc.tile_pool(name="big", bufs=2) as big,
            tc.tile_pool(name="psum", bufs=2, space="PSUM") as psum,
            tc.tile_pool(name="outp", bufs=3) as outp,
        ):
            # weight: [BPC, S] -> SBUF [128, BPC*KT]; column b*KT+k holds
            # w[b, k*128:(k+1)*128]
            w_sb = wpool.tile([P, BPC * KT], f32)
            nc.sync.dma_start(
                w_sb[:], w_dram.ap().rearrange("b (k p) -> p (b k)", p=P, k=KT)
            )

            for b in range(BPC):
                # per-batch operand tensors, [128, KT, D] bf16
                r16 = big.tile([P, KT, D], bf16, tag="r16")
                i16 = big.tile([P, KT, D], bf16, tag="i16")
                wr = big.tile([P, KT, D], bf16, tag="wr")
                wi = big.tile([P, KT, D], bf16, tag="wi")
                sa = big.tile([P, KT, D], bf16, tag="sa")
                sb_ = big.tile([P, KT, D], bf16, tag="sb")

                for k in range(KT):
                    r32 = stage.tile([P, D], f32, tag="r32")
                    i32 = stage.tile([P, D], f32, tag="i32")
                    nc.sync.dma_start(r32[:], r_dram[b, k * P:(k + 1) * P, :])
                    nc.sync.dma_start(i32[:], i_dram[b, k * P:(k + 1) * P, :])

                    # fp32 -> bf16 casts on the scalar (ACT) engine
                    nc.scalar.copy(r16[:, k, :], r32[:])
                    nc.scalar.copy(i16[:, k, :], i32[:])

                    wcol = w_sb[:, b * KT + k: b * KT + k + 1]
                    # WR = w * R, WI = w * I  (per-partition scalar multiply)
                    nc.vector.tensor_scalar_mul(wr[:, k, :], r16[:, k, :], wcol)
                    nc.vector.tensor_scalar_mul(wi[:, k, :], i16[:, k, :], wcol)
                    # SA = WR + WI, SB = R - I
                    nc.vector.tensor_add(sa[:, k, :], wr[:, k, :], wi[:, k, :])
                    nc.vector.tensor_sub(sb_[:, k, :], r16[:, k, :], i16[:, k, :])

                for m in range(MT):
                    ms = slice(m * P, (m + 1) * P)
                    for n in range(NT):
                        ns = slice(n * NW, (n + 1) * NW)
                        p1 = psum.tile([P, NW], f32, tag="p1")
                        q2 = psum.tile([P, NW], f32, tag="q2")
                        p3 = psum.tile([P, NW], f32, tag="p3")
                        for k in range(KT):
                            nc.tensor.matmul(p1[:], wr[:, k, ms], r16[:, k, ns],
                                             start=(k == 0), stop=(k == KT - 1))
                        for k in range(KT):
                            nc.tensor.matmul(q2[:], wi[:, k, ms], i16[:, k, ns],
                                             start=(k == 0), stop=(k == KT - 1))
                        for k in range(KT):
                            nc.tensor.matmul(p3[:], sa[:, k, ms], sb_[:, k, ns],
                                             start=(k == 0), stop=(k == KT - 1))

                        # DVE reads at most one PSUM operand per instruction:
                        # stage P1 into SBUF via the scalar engine first.
                        c1_t = outp.tile([P, NW], f32, tag="c1_t")
                        or_t = outp.tile([P, NW], f32, tag="or_t")
                        ti_t = outp.tile([P, NW], f32, tag="ti_t")
                        oi_t = outp.tile([P, NW], f32, tag="oi_t")
                        nc.scalar.copy(c1_t[:], p1[:])
                        nc.vector.tensor_add(or_t[:], c1_t[:], q2[:])
                        nc.vector.tensor_sub(ti_t[:], p3[:], c1_t[:])
                        nc.vector.tensor_add(oi_t[:], ti_t[:], q2[:])
                        nc.sync.dma_start(or_dram[b, ms, ns], or_t[:])
                        nc.sync.dma_start(oi_dram[b, ms, ns], oi_t[:])

    nc.compile()
    return nc


def _get_program():
    global _PROGRAM
    if _PROGRAM is None:
        _PROGRAM = _build_program()
    return _PROGRAM


def kernel(input_real, input_imag, weight, _spmd_kwargs=None):
    input_real = np.ascontiguousarray(input_real, dtype=np.float32)
    input_imag = np.ascontiguousarray(input_imag, dtype=np.float32)
    weight = np.ascontiguousarray(weight, dtype=np.float32)

    from concourse.bass_utils import run_bass_kernel_spmd

    nc = _get_program()
    in_maps = []
    for c in range(N_CORES):
        lo, hi = c * BPC, (c + 1) * BPC
        in_maps.append({
            "input_real": input_real[lo:hi],
            "input_imag": input_imag[lo:hi],
            "weight": weight[lo:hi],
        })
    res = run_bass_kernel_spmd(nc, in_maps, list(range(N_CORES)),
                               **(_spmd_kwargs or {}))
    out_r = np.concatenate([res.results[c]["out_r"] for c in range(N_CORES)], 0)
    out_i = np.concatenate([res.results[c]["out_i"] for c in range(N_CORES)], 0)
    kernel.last_results = res
    return (out_r, out_i)


# revision 7
# speedup vs baseline: 1.0524x; 1.0524x over previous
"""Trainium2 Bass kernel for the ComplexMixture density-matrix problem.

Math (per batch b), with R = input_real[b] [S, D], I = input_imag[b] [S, D],
w = weight[b] [S]:
    out_r[b] = R^T diag(w) R + I^T diag(w) I      (symmetric)
    out_i[b] = I^T diag(w) R - R^T diag(w) I      (antisymmetric)
Contraction is over S, which maps directly onto the PE array's partition
(K) dimension -- no input transposes needed.

Kernel algorithm:
  * 3-multiplication (Karatsuba/Gauss) complex product:
        P1 = WR^T @ R,  Q2 = WI^T @ I,  P3 = (WR+WI)^T @ (R-I)
        out_r = P1 + Q2
        out_i = P3 - P1 + Q2
    with WR = w*R, WI = w*I (3 big matmuls per batch instead of 4).
  * Hermitian symmetry: only the upper-triangular 128-row strips of the
    outputs are computed on the PE (58% of the matmul work); the lower
    triangle is filled by PE-transposing the computed 128x128 tiles
    (negated for out_i).
  * bf16 operands, fp32 PSUM accumulation (bf16 matmul is 4x fp32 rate).

Sharding: data-parallel over batch B=16 across 8 NeuronCores (2 per core),
no collectives.
"""

import sys

if "/opt/trn_rl_repo" not in sys.path:
    sys.path.insert(0, "/opt/trn_rl_repo")

import numpy as np

# Problem constants (hardcoded per harness contract)
B, S, D = 16, 1024, 768
N_CORES = 8
BPC = B // N_CORES  # batches per core
P = 128
KT = S // P   # 8 k-tiles along S
JT = D // P   # 6 column tiles of 128 along D

# Upper-triangular strip blocking: per row-strip m, computed column range
# [m*128, D) split into PSUM-bank-sized blocks (<=512 fp32).
def _strip_blocks(m):
    c0 = m * P
    width = D - c0
    blocks = []
    while width > 0:
        w = min(512, width)
        if width - w == 128 and w == 512:
            w = 384  # keep remainder >= 256 where possible
        blocks.append((c0, w))
        c0 += w
        width -= w
    return blocks


_PROGRAM = None


def _build_program():
    import concourse.mybir as mybir
    import concourse.tile as tile
    from concourse import bacc
    from concourse.masks import make_identity

    f32 = mybir.dt.float32
    bf16 = mybir.dt.bfloat16
    Alu = mybir.AluOpType

    nc = bacc.Bacc("TRN2", target_bir_lowering=False, debug=False,
                   num_devices=N_CORES)

    r_dram = nc.dram_tensor("input_real", [BPC, S, D], f32, kind="ExternalInput")
    i_dram = nc.dram_tensor("input_imag", [BPC, S, D], f32, kind="ExternalInput")
    w_dram = nc.dram_tensor("weight", [BPC, S], f32, kind="ExternalInput")
    or_dram = nc.dram_tensor("out_r", [BPC, D, D], f32, kind="ExternalOutput")
    oi_dram = nc.dram_tensor("out_i", [BPC, D, D], f32, kind="ExternalOutput")

    # DRAM views with S split into (k, p)
    r_kp = r_dram.ap().rearrange("b (k p) d -> b p k d", p=P)
    i_kp = i_dram.ap().rearrange("b (k p) d -> b p k d", p=P)

    with tile.TileContext(nc) as tc:
        with (
            tc.tile_pool(name="const", bufs=1) as const_pool,
            tc.tile_pool(name="stage", bufs=3) as stage,
            tc.tile_pool(name="big", bufs=2) as big,
            tc.tile_pool(name="psum", bufs=2, space="PSUM") as psum,
            tc.tile_pool(name="psum_t", bufs=2, space="PSUM") as psum_t,
            tc.tile_pool(name="outp", bufs=2) as outp,
            tc.tile_pool(name="mirr", bufs=2) as mirr,
        ):
            # weight: [BPC, S] -> SBUF [128, BPC*KT]; column b*KT+k holds
            # w[b, k*128:(k+1)*128]
            w_sb = const_pool.tile([P, BPC * KT], f32)
            nc.sync.dma_start(
                w_sb[:], w_dram.ap().rearrange("b (k p) -> p (b k)", p=P, k=KT)
            )
            ident = const_pool.tile([P, P], f32)
            make_identity(nc, ident[:])

            KC = 2  # k-tiles per input DMA chunk
            for b in range(BPC):
                r16 = big.tile([P, KT, D], bf16, tag="r16")
                i16 = big.tile([P, KT, D], bf16, tag="i16")
                wr = big.tile([P, KT, D], bf16, tag="wr")
                wi = big.tile([P, KT, D], bf16, tag="wi")
                sa = big.tile([P, KT, D], bf16, tag="sa")
                sb_ = big.tile([P, KT, D], bf16, tag="sb")

                for kc in range(KT // KC):
                    ks = slice(kc * KC, (kc + 1) * KC)
                    r32 = stage.tile([P, KC, D], f32, tag="r32")
                    i32 = stage.tile([P, KC, D], f32, tag="i32")
                    nc.sync.dma_start(r32[:], r_kp[b, :, ks, :])
                    nc.sync.dma_start(i32[:], i_kp[b, :, ks, :])
                    for dk in range(KC):
                        k = kc * KC + dk
                        # fp32 -> bf16 casts on the scalar (ACT) engine.
                        # i16 holds -I so SB = R - I becomes an ADD (the only
                        # 2-input ALU op GpSimd codegen supports here).
                        nc.scalar.copy(r16[:, k, :], r32[:, dk, :])
                        nc.scalar.mul(i16[:, k, :], i32[:, dk, :], -1.0)
                        wcol = w_sb[:, b * KT + k: b * KT + k + 1]
                        # WR = w*R, WIn = w*(-I) = -WI on DVE
                        nc.vector.tensor_scalar_mul(wr[:, k, :], r16[:, k, :], wcol)
                        nc.vector.tensor_scalar_mul(wi[:, k, :], i16[:, k, :], wcol)
                        # SA = WR + WI = wr - win (DVE); SB = R - I (GpSimd add)
                        nc.vector.tensor_sub(sa[:, k, :], wr[:, k, :], wi[:, k, :])
                        nc.gpsimd.tensor_add(sb_[:, k, :], r16[:, k, :], i16[:, k, :])

                for m in range(JT):
                    ms = slice(m * P, (m + 1) * P)
                    # mirror column-strip for DRAM rows (m+1)*128..D, col
                    # block m: accumulates the transposed off-diag tiles
                    nj = JT - 1 - m
                    if nj > 0:
                        mr_t = mirr.tile([P, nj, P], f32, tag="mr")
                        mi_t = mirr.tile([P, nj, P], f32, tag="mi")
                    for (c0, W) in _strip_blocks(m):
                        cs = slice(c0, c0 + W)
                        p1 = psum.tile([P, W], f32, tag="p1")
                        q2 = psum.tile([P, W], f32, tag="q2")
                        p3 = psum.tile([P, W], f32, tag="p3")
                        for k in range(KT):
                            nc.tensor.matmul(p1[:], wr[:, k, ms], r16[:, k, cs],
                                             start=(k == 0), stop=(k == KT - 1))
                        for k in range(KT):
                            nc.tensor.matmul(q2[:], wi[:, k, ms], i16[:, k, cs],
                                             start=(k == 0), stop=(k == KT - 1))
                        for k in range(KT):
                            nc.tensor.matmul(p3[:], sa[:, k, ms], sb_[:, k, cs],
                                             start=(k == 0), stop=(k == KT - 1))

                        # combine (DVE reads at most one PSUM operand per op)
                        c1_t = outp.tile([P, W], f32, tag="c1_t")
                        or_t = outp.tile([P, W], f32, tag="or_t")
                        ti_t = outp.tile([P, W], f32, tag="ti_t")
                        oi_t = outp.tile([P, W], f32, tag="oi_t")
                        nc.scalar.copy(c1_t[:], p1[:])
                        nc.vector.tensor_add(or_t[:], c1_t[:], q2[:])
                        nc.vector.tensor_sub(ti_t[:], p3[:], c1_t[:])
                        nc.vector.tensor_add(oi_t[:], ti_t[:], q2[:])
                        nc.sync.dma_start(or_dram[b, ms, cs], or_t[:])
                        nc.sync.dma_start(oi_dram[b, ms, cs], oi_t[:])

                        # transpose off-diagonal 128x128 tiles into the
                        # mirror strips of this column block's rows
                        j0 = max(c0 // P, m + 1)
                        for j in range(j0, (c0 + W) // P):
                            off = j * P - c0
                            tr = psum_t.tile([P, P], f32, tag="tr")
                            nc.tensor.transpose(tr[:], or_t[:, off:off + P],
                                                ident[:])
                            nc.scalar.copy(mr_t[:, j - m - 1, :], tr[:])
                            ti2 = psum_t.tile([P, P], f32, tag="tr")
                            nc.tensor.transpose(ti2[:], oi_t[:, off:off + P],
                                                ident[:])
                            # negate during PSUM->SBUF move (DVE)
                            nc.vector.tensor_scalar_mul(
                                mi_t[:, j - m - 1, :], ti2[:], -1.0)

                    # flush this strip's mirrors (SWDGE queue; sync does loads)
                    if nj > 0:
                        rows = slice((m + 1) * P, D)
                        cview_r = or_dram[b, rows, ms].rearrange(
                            "(j p) r -> p j r", p=P)
                        cview_i = oi_dram[b, rows, ms].rearrange(
                            "(j p) r -> p j r", p=P)
                        nc.gpsimd.dma_start(cview_r, mr_t[:])
                        nc.gpsimd.dma_start(cview_i, mi_t[:])

    nc.compile()
    return nc


def _get_program():
    global _PROGRAM
    if _PROGRAM is None:
        _PROGRAM = _build_program()
    return _PROGRAM


def kernel(input_real, input_imag, weight, _spmd_kwargs=None):
    input_real = np.ascontiguousarray(input_real, dtype=np.float32)
    input_imag = np.ascontiguousarray(input_imag, dtype=np.float32)
    weight = np.ascontiguousarray(weight, dtype=np.float32)

    from concourse.bass_utils import run_bass_kernel_spmd

    nc = _get_program()
    in_maps = []
    for c in range(N_CORES):
        lo, hi = c * BPC, (c + 1) * BPC
        in_maps.append({
            "input_real": input_real[lo:hi],
            "input_imag": input_imag[lo:hi],
            "weight": weight[lo:hi],
        })
    res = run_bass_kernel_spmd(nc, in_maps, list(range(N_CORES)),
                               **(_spmd_kwargs or {}))
    out_r = np.concatenate([res.results[c]["out_r"] for c in range(N_CORES)], 0)
    out_i = np.concatenate([res.results[c]["out_i"] for c in range(N_CORES)], 0)
    kernel.last_results = res
    return (out_r, out_i)


# revision 11
# speedup vs baseline: 1.1641x; 1.1062x over previous
"""Trainium2 Bass kernel for the ComplexMixture density-matrix problem.

Math (per batch b), with R = input_real[b] [S, D], I = input_imag[b] [S, D],
w = weight[b] [S]:
    out_r[b] = R^T diag(w) R + I^T diag(w) I      (symmetric)
    out_i[b] = I^T diag(w) R - R^T diag(w) I      (antisymmetric)
Contraction is over S, which maps directly onto the PE array's partition
(K) dimension -- no input transposes needed.

Kernel algorithm:
  * 3-multiplication (Karatsuba/Gauss) complex product:
        P1 = WR^T @ R,  Q2 = WI^T @ I,  P3 = (WR+WI)^T @ (R-I)
        out_r = P1 + Q2
        out_i = P3 - P1 + Q2
    with WR = w*R, WI = w*I (3 big matmuls per batch instead of 4).
    Internally the imag operands are stored negated (In = -I, WIn = -WI)
    which changes no matmul result that we use:
        WIn^T @ In = WI^T I = Q2,  SB = R - I = r + In,  SA = WR+WI = wr-WIn.
  * Hermitian symmetry: only the upper-triangular 128-row strips of the
    outputs are computed on the PE (58% of the matmul work); the lower
    triangle is filled by PE-transposing the computed 128x128 tiles
    (negated for out_i).  Transposes are emitted one block late so they
    never head-of-line-block the next block's matmuls in the PE queue.
  * bf16 operands, fp32 PSUM accumulation (bf16 matmul is 4x fp32 rate).

Sharding: data-parallel over batch B=16 across 8 NeuronCores (2 per core),
no collectives.
"""

import sys

if "/opt/trn_rl_repo" not in sys.path:
    sys.path.insert(0, "/opt/trn_rl_repo")

import numpy as np

# Problem constants (hardcoded per harness contract)
B, S, D = 16, 1024, 768
N_CORES = 8
BPC = B // N_CORES  # batches per core
P = 128
KT = S // P   # 8 k-tiles along S
JT = D // P   # 6 column tiles of 128 along D


def _strip_blocks(m):
    """Upper-triangular strip m: computed column range [m*128, D) split
    into PSUM-bank-sized blocks (<=512 fp32)."""
    c0 = m * P
    width = D - c0
    blocks = []
    while width > 0:
        w = min(512, width)
        if width - w == 128 and w == 512:
            w = 384  # keep remainder >= 256 where possible
        blocks.append((c0, w))
        c0 += w
        width -= w
    return blocks


_PROGRAM = None


def _build_program():
    import concourse.mybir as mybir
    import concourse.tile as tile
    from concourse import bacc
    from concourse.masks import make_identity

    f32 = mybir.dt.float32
    bf16 = mybir.dt.bfloat16

    nc = bacc.Bacc("TRN2", target_bir_lowering=False, debug=False,
                   num_devices=N_CORES)

    r_dram = nc.dram_tensor("input_real", [BPC, S, D], f32, kind="ExternalInput")
    i_dram = nc.dram_tensor("input_imag", [BPC, S, D], f32, kind="ExternalInput")
    w_dram = nc.dram_tensor("weight", [BPC, S], f32, kind="ExternalInput")
    or_dram = nc.dram_tensor("out_r", [BPC, D, D], f32, kind="ExternalOutput")
    oi_dram = nc.dram_tensor("out_i", [BPC, D, D], f32, kind="ExternalOutput")

    # DRAM views with S split into (k, p)
    r_kp = r_dram.ap().rearrange("b (k p) d -> b p k d", p=P)
    i_kp = i_dram.ap().rearrange("b (k p) d -> b p k d", p=P)

    with tile.TileContext(nc) as tc:
        with (
            tc.tile_pool(name="const", bufs=1) as const_pool,
            tc.tile_pool(name="stage", bufs=2) as stage,
            tc.tile_pool(name="big", bufs=2) as big,
            tc.tile_pool(name="psum", bufs=2, space="PSUM") as psum,
            tc.tile_pool(name="psum_t", bufs=2, space="PSUM") as psum_t,
            tc.tile_pool(name="outp", bufs=3) as outp,
            tc.tile_pool(name="mirr", bufs=2) as mirr,
        ):
            # weight: [BPC, S] -> SBUF [128, BPC*KT]; column b*KT+k holds
            # w[b, k*128:(k+1)*128]
            w_sb = const_pool.tile([P, BPC * KT], f32)
            nc.sync.dma_start(
                w_sb[:], w_dram.ap().rearrange("b (k p) -> p (b k)", p=P, k=KT)
            )
            ident = const_pool.tile([P, P], f32)
            make_identity(nc, ident[:])

            KC = 2  # k-tiles per input DMA chunk

            def emit_prep(b, ops):
                """loads + elementwise prep for one batch; returns operand set"""
                r16 = big.tile([P, KT, D], bf16, tag="r16")
                i16 = big.tile([P, KT, D], bf16, tag="i16")  # holds -I
                wr = big.tile([P, KT, D], bf16, tag="wr")
                wi = big.tile([P, KT, D], bf16, tag="wi")    # holds -WI
                sa = big.tile([P, KT, D], bf16, tag="sa")    # WR + WI
                sb_ = big.tile([P, KT, D], bf16, tag="sb")   # R - I
                stages = []
                for kc in range(KT // KC):
                    ks = slice(kc * KC, (kc + 1) * KC)
                    r32 = stage.tile([P, KC, D], f32, tag="r32")
                    i32 = stage.tile([P, KC, D], f32, tag="i32")
                    nc.sync.dma_start(r32[:], r_kp[b, :, ks, :])
                    nc.sync.dma_start(i32[:], i_kp[b, :, ks, :])
                    stages.append((r32, i32))
                for kc in range(KT // KC):
                    r32, i32 = stages[kc]
                    for dk in range(KC):
                        k = kc * KC + dk
                        # casts: r on DVE, negated i on ACT
                        nc.vector.tensor_copy(r16[:, k, :], r32[:, dk, :])
                        nc.scalar.mul(i16[:, k, :], i32[:, dk, :], -1.0)
                        wcol = w_sb[:, b * KT + k: b * KT + k + 1]
                        nc.vector.tensor_scalar_mul(wr[:, k, :], r16[:, k, :], wcol)
                        nc.vector.tensor_scalar_mul(wi[:, k, :], i16[:, k, :], wcol)
                        nc.vector.tensor_sub(sa[:, k, :], wr[:, k, :], wi[:, k, :])
                        nc.vector.tensor_add(sb_[:, k, :], r16[:, k, :], i16[:, k, :])
                ops[b] = (r16, i16, wr, wi, sa, sb_)

            pending = []  # deferred transpose/flush emitters

            def emit_pending():
                for fn in pending:
                    fn()
                pending.clear()

            def emit_groups(b, ops):
                r16, i16, wr, wi, sa, sb_ = ops[b]
                for m in range(JT):
                    ms = slice(m * P, (m + 1) * P)
                    nj = JT - 1 - m
                    if nj > 0:
                        mr_t = mirr.tile([P, nj, P], f32, tag="mr")
                        mi_t = mirr.tile([P, nj, P], f32, tag="mi")
                    blocks = _strip_blocks(m)
                    for bi, (c0, W) in enumerate(blocks):
                        cs = slice(c0, c0 + W)
                        p1 = psum.tile([P, W], f32, tag="p1")
                        q2 = psum.tile([P, W], f32, tag="q2")
                        p3 = psum.tile([P, W], f32, tag="p3")
                        for k in range(KT):
                            nc.tensor.matmul(p1[:], wr[:, k, ms], r16[:, k, cs],
                                             start=(k == 0), stop=(k == KT - 1))
                        for k in range(KT):
                            nc.tensor.matmul(q2[:], wi[:, k, ms], i16[:, k, cs],
                                             start=(k == 0), stop=(k == KT - 1))
                        for k in range(KT):
                            nc.tensor.matmul(p3[:], sa[:, k, ms], sb_[:, k, cs],
                                             start=(k == 0), stop=(k == KT - 1))

                        # combine (DVE reads at most one PSUM operand per op)
                        c1_t = outp.tile([P, W], f32, tag="c1_t")
                        or_t = outp.tile([P, W], f32, tag="or_t")
                        ti_t = outp.tile([P, W], f32, tag="ti_t")
                        oi_t = outp.tile([P, W], f32, tag="oi_t")
                        nc.scalar.copy(c1_t[:], p1[:])
                        nc.vector.tensor_add(or_t[:], c1_t[:], q2[:])
                        nc.vector.tensor_sub(ti_t[:], p3[:], c1_t[:])
                        nc.vector.tensor_add(oi_t[:], ti_t[:], q2[:])
                        nc.sync.dma_start(or_dram[b, ms, cs], or_t[:])
                        nc.sync.dma_start(oi_dram[b, ms, cs], oi_t[:])

                        # previous block's transposes land in the PE queue
                        # behind this block's matmuls (no head-of-line stall)
                        emit_pending()

                        def mk_transposes(m=m, c0=c0, W=W, or_t=or_t,
                                          oi_t=oi_t, mr_t=mr_t if nj else None,
                                          mi_t=mi_t if nj else None,
                                          last=(bi == len(blocks) - 1), b=b):
                            j0 = max(c0 // P, m + 1)
                            for j in range(j0, (c0 + W) // P):
                                off = j * P - c0
                                tr = psum_t.tile([P, P], f32, tag="tr")
                                nc.tensor.transpose(tr[:], or_t[:, off:off + P],
                                                    ident[:])
                                nc.scalar.copy(mr_t[:, j - m - 1, :], tr[:])
                                ti2 = psum_t.tile([P, P], f32, tag="tr")
                                nc.tensor.transpose(ti2[:], oi_t[:, off:off + P],
                                                    ident[:])
                                nc.scalar.mul(mi_t[:, j - m - 1, :], ti2[:], -1.0)
                            if last and mr_t is not None:
                                rows = slice((m + 1) * P, D)
                                ms2 = slice(m * P, (m + 1) * P)
                                cview_r = or_dram[b, rows, ms2].rearrange(
                                    "(j p) r -> p j r", p=P)
                                cview_i = oi_dram[b, rows, ms2].rearrange(
                                    "(j p) r -> p j r", p=P)
                                nc.sync.dma_start(cview_r, mr_t[:])
                                nc.sync.dma_start(cview_i, mi_t[:])

                        pending.append(mk_transposes)
                emit_pending()

            ops = {}
            for b in range(BPC):
                emit_prep(b, ops)
            for b in range(BPC):
                emit_groups(b, ops)

    nc.compile()
    return nc


def _get_program():
    global _PROGRAM
    if _PROGRAM is None:
        _PROGRAM = _build_program()
    return _PROGRAM


def kernel(input_real, input_imag, weight, _spmd_kwargs=None):
    input_real = np.ascontiguousarray(input_real, dtype=np.float32)
    input_imag = np.ascontiguousarray(input_imag, dtype=np.float32)
    weight = np.ascontiguousarray(weight, dtype=np.float32)

    from concourse.bass_utils import run_bass_kernel_spmd

    nc = _get_program()
    in_maps = []
    for c in range(N_CORES):
        lo, hi = c * BPC, (c + 1) * BPC
        in_maps.append({
            "input_real": input_real[lo:hi],
            "input_imag": input_imag[lo:hi],
            "weight": weight[lo:hi],
        })
    res = run_bass_kernel_spmd(nc, in_maps, list(range(N_CORES)),
                               **(_spmd_kwargs or {}))
    out_r = np.concatenate([res.results[c]["out_r"] for c in range(N_CORES)], 0)
    out_i = np.concatenate([res.results[c]["out_i"] for c in range(N_CORES)], 0)
    kernel.last_results = res
    return (out_r, out_i)
